# revision 38
# baseline (speedup 1.0000x reference)
"""Trainium2 kernel for nn_Phngb_38474317037901 (retrieval_knn).

reference:
    dist  = euclidean_distances(coordinates.T)          # [F, F], F=4096
    nbr   = top_k(-dist, 8).indices                     # [F, 8]
    out   = concat([inputs[:, :1], inputs[:, nbr.flat]], axis=1)[:, None, :, None]

negkey[p, j] = 2*c_p . x_j - |x_j|^2 is order-equivalent to -dist per row.
The matmul contracts only the 64 coord rows (one PE pass; a 65th row
doubled PE instruction count); -sq[j] (host-computed f32, replicated to
128 partitions) is subtracted by DVE scalar_tensor_tensor during the
PSUM->SBUF move.  Margins verified: min 8/9 boundary gap 8.8e-4 >> f32
rounding ~1e-5, so f32 topk matches the f64/f32 reference ordering.

Device strategy (8 cores, SPMD, output-row sharding in transposed space):
  - in_t = inputs.T  [F, B] as float16 replicated to every core (DRAM
    resident).  The gather/store payload is fp16 (halves HBM traffic vs
    f32; quantization rel-err ~1e-4 << 2e-2 gate); topk stays exact f32.
  - Core c owns features f in [512c, 512c+512): computes negkey = 2*G - sq_j
    (order-equivalent to -dist per row) via PE matmuls, takes top-8 per row
    with vector max/max_index, then gathers the 4096 neighbor rows of in_t
    with indirect DMA (offset AP = i8[:, k] directly, no DRAM round-trip)
    and streams them to out_g [4, 128, 8, B] (flat row m = 1024q + 8p + k).
  - k=0 shortcut: dist[p,p]=0 exactly, so nbr[p,0]==p always (no tie is
    possible: the nearest distinct point is ~8 away).  The k=0 output
    columns are therefore inputs verbatim; the host fills them in exact
    f32 (out[:, 1::8] = inputs) and the device only gathers k=1..7 —
    56MB instead of 64MB moved per core.
  - Stores alternate between the two HWDGE rings (SP + ACT): one ring caps
    at ~210 GB/s and paces the whole pipeline; split, the core sustains
    ~410-427 GB/s combined gather-read + store-write (per-NC HBM mix cap).
  - gat pool bufs=4 (not 8): it couples the gather stream to the store
    stream, so reads cannot front-run and build a write backlog that
    drains at the slower solo-write rate (~30us tail with bufs=8).
  - Host stitches: out[:, 0] = inputs[:, 0]; out[:, 1:] = concat(out_g).T
    (fp16 upcast to f32 on assignment).
"""

import sys

import numpy as np

for _p in ("/opt/trn_rl_repo",):
    if _p not in sys.path:
        sys.path.insert(0, _p)

B = 4096        # batch
F = 4096        # features (points)
D = 64          # coordinate dim (single PE pass; -sq folded post-matmul)
K = 8           # neighbors
NCORES = 8
FPC = F // NCORES            # features per core (512)
MPC = FPC * K                # output rows per core (4096)

LAST_RESULTS = None          # BassKernelResults of the most recent run (for test harness)


def _build_nc():
    import concourse.bacc as bacc
    import concourse.bass as bass
    import concourse.mybir as mybir
    import concourse.tile as tile

    f32 = mybir.dt.float32
    f16 = mybir.dt.float16
    u32 = mybir.dt.uint32

    # Bacc (not plain Bass): its compile() runs generate_event_semaphores,
    # which legalizes the TRN2 1-sync-wait-per-instruction limit.
    nc = bacc.Bacc("TRN2", target_bir_lowering=False)

    # coordsx host layout [64, F+FPC]:
    #   cols 0..F-1:      coords (matmul rhs)
    #   cols F..F+FPC-1:  2*coords_mine (matmul lhsT)
    # 64 contraction rows => ONE PE pass per matmul (65 rows forced two).
    coordsx = nc.dram_tensor("coordsx", [D, F + FPC], f32, kind="ExternalInput")
    # sq[j] replicated to 128 partitions; subtracted during PSUM->SBUF copy.
    sqb = nc.dram_tensor("sqb", [128, F], f32, kind="ExternalInput")
    # fp16 payload: gather+store traffic halves; topk stays exact f32.
    in_t = nc.dram_tensor("in_t", [F, B], f16, kind="ExternalInput")
    # k=1..7 only: dist[p,p]=0 exactly => nbr[p,0]==p always (no tie is
    # possible; nearest distinct point is ~8 away), so the k=0 output
    # column is inputs[:, p] verbatim and the HOST fills it in exact f32.
    # The device moves 56MB instead of 64MB per core.
    out_g = nc.dram_tensor(
        "out_g", [FPC // 128, K - 1, 128, B], f16, kind="ExternalOutput"
    )

    with tile.TileContext(nc) as tc:
        with (
            tc.tile_pool(name="const", bufs=1) as constp,
            tc.tile_pool(name="nk", bufs=2) as nkp,
            tc.tile_pool(name="ps", bufs=4, space="PSUM") as psp,
            tc.tile_pool(name="small", bufs=8) as smallp,
            tc.tile_pool(name="gat", bufs=4) as gp,
        ):
            coords_sb = constp.tile([D, F + FPC], f32)
            sq_sb = constp.tile([128, F], f32)
            # split loads; sqb goes on the other HWDGE ring (scalar) in
            # 512-col blocks so sq_sb[j] lands before stt_j needs it.
            nc.sync.dma_start(out=coords_sb[:, F:], in_=coordsx[:, F:])
            for blk in range(4):
                cs = slice(1024 * blk, 1024 * (blk + 1))
                nc.sync.dma_start(out=coords_sb[:, cs], in_=coordsx[:, cs])
            for blk in range(8):
                cs = slice(512 * blk, 512 * (blk + 1))
                nc.scalar.dma_start(out=sq_sb[:, cs], in_=sqb[:, cs])

            for q in range(FPC // 128):
                # negkey[p, j] = 2*c_p . x_j - sq[j]; 64-row matmul + fused
                # subtract in the DVE move out of PSUM.
                nk = nkp.tile([128, F], f32)
                # hierarchical top-8: per-chunk top-8s overlap the PE
                # matmuls; only max8(cand) + one max_index sit on the
                # critical path after the last chunk.
                cand = smallp.tile([128, 8 * (F // 512)], f32)
                for j in range(F // 512):
                    js = slice(512 * j, 512 * (j + 1))
                    ps = psp.tile([128, 512], f32)
                    nc.tensor.matmul(
                        out=ps[:, :],
                        lhsT=coords_sb[:, F + 128 * q:F + 128 * (q + 1)],
                        rhs=coords_sb[:, js],
                        start=True, stop=True,
                    )
                    nc.vector.scalar_tensor_tensor(
                        out=nk[:, js], in0=ps[:, :], scalar=1.0,
                        in1=sq_sb[:, js],
                        op0=mybir.AluOpType.mult,
                        op1=mybir.AluOpType.subtract,
                    )
                    nc.vector.max(cand[:, 8 * j:8 * (j + 1)], nk[:, js])

                v8 = smallp.tile([128, K], f32)
                i8 = smallp.tile([128, K], u32)
                nc.vector.max(v8[:, :], cand[:, :])
                nc.vector.max_index(i8[:, :], v8[:, :], nk[:, :])

                for k in range(1, K):
                    gt = gp.tile([128, B], f16)
                    nc.gpsimd.indirect_dma_start(
                        out=gt[:, :],
                        out_offset=None,
                        in_=in_t[:, :],
                        in_offset=bass.IndirectOffsetOnAxis(
                            ap=i8[:, k:k + 1], axis=0
                        ),
                    )
                    # alternate the two HWDGE rings (SP + ACT): a single
                    # ring caps at ~210 GB/s and paces the whole pipeline.
                    # (k+q) parity balances rings given 7 gathers/chunk.
                    st_eng = nc.sync if (k + q) % 2 == 0 else nc.scalar
                    st_eng.dma_start(out=out_g[q, k - 1, :, :], in_=gt[:, :])

    nc.compile()
    return nc


def kernel(inputs: np.ndarray, coordinates: np.ndarray) -> np.ndarray:
    global LAST_RESULTS
    from concourse.bass_utils import run_bass_kernel_spmd

    inputs = np.ascontiguousarray(np.asarray(inputs, dtype=np.float32))
    coords = np.ascontiguousarray(np.asarray(coordinates, dtype=np.float32))

    nc = _build_nc()

    in_t = np.ascontiguousarray(inputs.T.astype(np.float16))
    sq = (coords * coords).sum(axis=0, dtype=np.float32)
    sqb = np.ascontiguousarray(np.tile(sq[None, :], (128, 1)))
    in_maps = []
    for c in range(NCORES):
        mine = 2.0 * coords[:, FPC * c:FPC * (c + 1)]          # [64, FPC]
        cx = np.concatenate([coords, mine], axis=1)
        in_maps.append(
            {"coordsx": np.ascontiguousarray(cx), "in_t": in_t, "sqb": sqb}
        )

    res = run_bass_kernel_spmd(nc, in_maps, list(range(NCORES)))
    LAST_RESULTS = res

    out = np.empty((B, 1 + F * K), dtype=np.float32)
    out[:, 0] = inputs[:, 0]
    # k=0 neighbor is always self (dist diagonal is exactly 0): exact f32
    # copy from inputs, no device traffic.
    out[:, 1::8] = inputs
    for c in range(NCORES):
        # [4, K-1, 128, B] -> [(q,p)=FPC, K-1, B]
        arr = np.asarray(res.results[c]["out_g"]).reshape(
            FPC // 128, K - 1, 128, B
        )
        blk3 = arr.transpose(0, 2, 1, 3).reshape(FPC, K - 1, B)
        for k in range(1, K):
            out[:, 1 + MPC * c + k:1 + MPC * (c + 1):K] = blk3[:, k - 1, :].T
    return out[:, None, :, None]



# revision 39
# speedup vs baseline: 1.1306x; 1.1306x over previous
"""Trainium2 kernel for nn_Phngb_38474317037901 (retrieval_knn).

reference:
    dist  = euclidean_distances(coordinates.T)          # [F, F], F=4096
    nbr   = top_k(-dist, 8).indices                     # [F, 8]
    out   = concat([inputs[:, :1], inputs[:, nbr.flat]], axis=1)[:, None, :, None]

negkey[p, j] = 2*c_p . x_j - |x_j|^2 is order-equivalent to -dist per row.
The matmul contracts only the 64 coord rows (one PE pass; a 65th row
doubled PE instruction count); -sq[j] (host-computed f32, replicated to
128 partitions) is subtracted by DVE scalar_tensor_tensor during the
PSUM->SBUF move.  Margins verified: min 8/9 boundary gap 8.8e-4 >> f32
rounding ~1e-5, so f32 topk matches the f64/f32 reference ordering.

Device strategy (8 cores, SPMD, output-row sharding in transposed space):
  - in_t = inputs.T  [F, B] as float16 replicated to every core (DRAM
    resident).  The gather/store payload is fp16 (halves HBM traffic vs
    f32; quantization rel-err ~1e-4 << 2e-2 gate); topk stays exact f32.
  - Core c owns features f in [512c, 512c+512): computes negkey = 2*G - sq_j
    (order-equivalent to -dist per row) via PE matmuls, takes top-8 per row
    with vector max/max_index, then gathers the 4096 neighbor rows of in_t
    with indirect DMA (offset AP = i8[:, k] directly, no DRAM round-trip)
    and streams them to out_g [4, 128, 8, B] (flat row m = 1024q + 8p + k).
  - k=0 shortcut: dist[p,p]=0 exactly, so nbr[p,0]==p always (no tie is
    possible: the nearest distinct point is ~8 away).  The k=0 output
    columns are therefore inputs verbatim; the host fills them in exact
    f32 (out[:, 1::8] = inputs) and the device only gathers k=1..7 —
    56MB instead of 64MB moved per core.
  - Stores alternate between the two HWDGE rings (SP + ACT): one ring caps
    at ~210 GB/s and paces the whole pipeline; split, the core sustains
    ~410-427 GB/s combined gather-read + store-write (per-NC HBM mix cap).
  - gat pool bufs=4 (not 8): it couples the gather stream to the store
    stream, so reads cannot front-run and build a write backlog that
    drains at the slower solo-write rate (~30us tail with bufs=8).
  - Host stitches: out[:, 0] = inputs[:, 0]; out[:, 1:] = concat(out_g).T
    (fp16 upcast to f32 on assignment).
"""

import sys

import numpy as np

for _p in ("/opt/trn_rl_repo",):
    if _p not in sys.path:
        sys.path.insert(0, _p)

B = 4096        # batch
F = 4096        # features (points)
D = 64          # coordinate dim (single PE pass; -sq folded post-matmul)
K = 8           # neighbors
NCORES = 8
FPC = F // NCORES            # features per core (512)
MPC = FPC * K                # output rows per core (4096)

LAST_RESULTS = None          # BassKernelResults of the most recent run (for test harness)


def _build_nc():
    import concourse.bacc as bacc
    import concourse.bass as bass
    import concourse.mybir as mybir
    import concourse.tile as tile

    f32 = mybir.dt.float32
    f16 = mybir.dt.float16
    u32 = mybir.dt.uint32

    # Bacc (not plain Bass): its compile() runs generate_event_semaphores,
    # which legalizes the TRN2 1-sync-wait-per-instruction limit.
    nc = bacc.Bacc("TRN2", target_bir_lowering=False)

    # coordsx host layout [64, F+FPC]:
    #   cols 0..F-1:      coords (matmul rhs)
    #   cols F..F+FPC-1:  2*coords_mine (matmul lhsT)
    # 64 contraction rows => ONE PE pass per matmul (65 rows forced two).
    coordsx = nc.dram_tensor("coordsx", [D, F + FPC], f32, kind="ExternalInput")
    # sq[j] replicated to 128 partitions; subtracted during PSUM->SBUF copy.
    sqb = nc.dram_tensor("sqb", [128, F], f32, kind="ExternalInput")
    # fp16 payload: gather+store traffic halves; topk stays exact f32.
    in_t = nc.dram_tensor("in_t", [F, B], f16, kind="ExternalInput")
    # k=1..7 only: dist[p,p]=0 exactly => nbr[p,0]==p always (no tie is
    # possible; nearest distinct point is ~8 away), so the k=0 output
    # column is inputs[:, p] verbatim and the HOST fills it in exact f32.
    # The device moves 56MB instead of 64MB per core.
    out_g = nc.dram_tensor(
        "out_g", [FPC // 128, K - 1, 128, B], f16, kind="ExternalOutput"
    )

    with tile.TileContext(nc) as tc:
        with (
            tc.tile_pool(name="const", bufs=1) as constp,
            tc.tile_pool(name="nk", bufs=2) as nkp,
            tc.tile_pool(name="ps", bufs=4, space="PSUM") as psp,
            tc.tile_pool(name="small", bufs=8) as smallp,
            tc.tile_pool(name="gat", bufs=4) as gp,
        ):
            coords_sb = constp.tile([D, F + FPC], f32)
            sq_sb = constp.tile([128, F], f32)
            # coords split across BOTH HWDGE rings: matmul0's release
            # tracks the LAST coords DMA (whole-tile dep), so finishing
            # all five sooner shifts the whole index chain left.  sqb
            # follows on the scalar ring in 512-col blocks.
            nc.sync.dma_start(out=coords_sb[:, F:], in_=coordsx[:, F:])
            for blk in range(4):
                cs = slice(1024 * blk, 1024 * (blk + 1))
                eng = nc.sync if blk < 2 else nc.scalar
                eng.dma_start(out=coords_sb[:, cs], in_=coordsx[:, cs])
            for blk in range(8):
                cs = slice(512 * blk, 512 * (blk + 1))
                nc.scalar.dma_start(out=sq_sb[:, cs], in_=sqb[:, cs])

            for q in range(FPC // 128):
                # negkey[p, j] = 2*c_p . x_j - sq[j]; 64-row matmul + fused
                # subtract in the DVE move out of PSUM.
                nk = nkp.tile([128, F], f32)
                # hierarchical top-8: per-chunk top-8s overlap the PE
                # matmuls; only max8(cand) + one max_index sit on the
                # critical path after the last chunk.
                cand = smallp.tile([128, 8 * (F // 512)], f32)
                for j in range(F // 512):
                    js = slice(512 * j, 512 * (j + 1))
                    ps = psp.tile([128, 512], f32)
                    nc.tensor.matmul(
                        out=ps[:, :],
                        lhsT=coords_sb[:, F + 128 * q:F + 128 * (q + 1)],
                        rhs=coords_sb[:, js],
                        start=True, stop=True,
                    )
                    nc.vector.scalar_tensor_tensor(
                        out=nk[:, js], in0=ps[:, :], scalar=1.0,
                        in1=sq_sb[:, js],
                        op0=mybir.AluOpType.mult,
                        op1=mybir.AluOpType.subtract,
                    )
                    nc.vector.max(cand[:, 8 * j:8 * (j + 1)], nk[:, js])

                v8 = smallp.tile([128, K], f32)
                i8 = smallp.tile([128, K], u32)
                nc.vector.max(v8[:, :], cand[:, :])
                nc.vector.max_index(i8[:, :], v8[:, :], nk[:, :])

                for k in range(1, K):
                    gt = gp.tile([128, B], f16)
                    nc.gpsimd.indirect_dma_start(
                        out=gt[:, :],
                        out_offset=None,
                        in_=in_t[:, :],
                        in_offset=bass.IndirectOffsetOnAxis(
                            ap=i8[:, k:k + 1], axis=0
                        ),
                    )
                    # alternate the two HWDGE rings (SP + ACT): a single
                    # ring caps at ~210 GB/s and paces the whole pipeline.
                    # (k+q) parity balances rings given 7 gathers/chunk.
                    st_eng = nc.sync if (k + q) % 2 == 0 else nc.scalar
                    st_eng.dma_start(out=out_g[q, k - 1, :, :], in_=gt[:, :])

    nc.compile()
    return nc


def kernel(inputs: np.ndarray, coordinates: np.ndarray) -> np.ndarray:
    global LAST_RESULTS
    from concourse.bass_utils import run_bass_kernel_spmd

    inputs = np.ascontiguousarray(np.asarray(inputs, dtype=np.float32))
    coords = np.ascontiguousarray(np.asarray(coordinates, dtype=np.float32))

    nc = _build_nc()

    in_t = np.ascontiguousarray(inputs.T.astype(np.float16))
    sq = (coords * coords).sum(axis=0, dtype=np.float32)
    sqb = np.ascontiguousarray(np.tile(sq[None, :], (128, 1)))
    in_maps = []
    for c in range(NCORES):
        mine = 2.0 * coords[:, FPC * c:FPC * (c + 1)]          # [64, FPC]
        cx = np.concatenate([coords, mine], axis=1)
        in_maps.append(
            {"coordsx": np.ascontiguousarray(cx), "in_t": in_t, "sqb": sqb}
        )

    res = run_bass_kernel_spmd(nc, in_maps, list(range(NCORES)))
    LAST_RESULTS = res

    out = np.empty((B, 1 + F * K), dtype=np.float32)
    out[:, 0] = inputs[:, 0]
    # k=0 neighbor is always self (dist diagonal is exactly 0): exact f32
    # copy from inputs, no device traffic.
    out[:, 1::8] = inputs
    for c in range(NCORES):
        # [4, K-1, 128, B] -> [(q,p)=FPC, K-1, B]
        arr = np.asarray(res.results[c]["out_g"]).reshape(
            FPC // 128, K - 1, 128, B
        )
        blk3 = arr.transpose(0, 2, 1, 3).reshape(FPC, K - 1, B)
        for k in range(1, K):
            out[:, 1 + MPC * c + k:1 + MPC * (c + 1):K] = blk3[:, k - 1, :].T
    return out[:, None, :, None]

